# revision 1
# baseline (speedup 1.0000x reference)
"""Trainium2 Bass kernel v7: Gauss 3-mult, 4x2 sharding, head+tail optimized.

v5 + graded startup DMA chunks, and a split partition-reduce: kt-pairs 0-2
accumulate |w|^2 into sqacc (vector, free), which is flushed into the PSUM
reduce bank by a ones-matmul that overlaps the last kt-pair's dense matmuls;
only the last pair's squares feed the PSUM reduce directly, so the tail
chain after the final big matmul is minimal.
"""

import numpy as np
import ml_dtypes
from contextlib import ExitStack

N_QUBITS = 12
DIM = 4096
HALF = 2048
B = 2048
NCORES = 8
NBS = 4
NKS = 2
BLOC = B // NBS             # 512
KLOC = HALF // NKS          # 1024
NT = DIM // 128             # 32
KT = KLOC // 128            # 8 k-tiles per core -> 4 kt-pairs
KTP = KT // 2
W_SCALE = 2.0 ** -7

_BUILT = None


def _host_prep(inputs, weight, entangle_matrix):
    x = np.asarray(inputs, dtype=np.float32)
    w = np.asarray(weight, dtype=np.float32)
    E = np.asarray(entangle_matrix, dtype=np.float32)

    ry = x / 2.0
    rz = (x * x) / 2.0
    a = np.cos(ry) * np.exp(-1j * rz)
    bq = np.sin(ry) * np.exp(1j * rz)
    col2 = np.stack([a, bq], axis=-1).astype(np.complex64)

    u = np.ones((B, 1), np.complex64)
    for q in range(N_QUBITS):
        u = (u[:, :, None] * col2[:, q][:, None, :]).reshape(B, -1)

    ur = np.ascontiguousarray(u.real)
    ui = np.ascontiguousarray(u.imag)

    wr = w[3:]
    tx = wr[:N_QUBITS] / 2.0
    tz = wr[N_QUBITS:] / 2.0
    c, s = np.cos(tx), np.sin(tx)
    rx = np.stack([np.stack([c, -1j * s], -1), np.stack([-1j * s, c], -1)], -2)
    ez = np.exp(-1j * tz)
    zz = np.zeros_like(ez)
    rzm = np.stack([np.stack([ez, zz], -1), np.stack([zz, np.exp(1j * tz)], -1)], -2)
    mats = np.einsum('qij,qjk->qik', rx, rzm)

    def kron_list(ms):
        M = ms[0]
        for m_ in ms[1:]:
            M = np.kron(M, m_)
        return M

    RA = kron_list([mats[q] for q in range(0, 5)]).astype(np.complex64)
    RB = kron_list([mats[q] for q in range(5, 12)]).astype(np.complex64)

    def ry2(t):
        a_ = t / 2.0
        return np.array([[np.cos(a_), -np.sin(a_)], [np.sin(a_), np.cos(a_)]],
                        dtype=np.float32)

    rot = ry2(w[2]) @ ry2(w[1]) @ ry2(w[0])
    Etil = rot[0, 0] * E[:HALF, :] + rot[0, 1] * E[HALF:, :]

    E3 = Etil.reshape(HALF, 32, 128)
    Tr = (E3.reshape(-1, 128) @ RB.real).reshape(HALF, 32, 128)
    Ti = (E3.reshape(-1, 128) @ RB.imag).reshape(HALF, 32, 128)
    RAr, RAi = RA.real.astype(np.float32), RA.imag.astype(np.float32)
    Gr = (np.einsum('khL,hH->kHL', Tr, RAr)
          - np.einsum('khL,hH->kHL', Ti, RAi)).reshape(HALF, DIM)
    Gi = (np.einsum('khL,hH->kHL', Tr, RAi)
          + np.einsum('khL,hH->kHL', Ti, RAr)).reshape(HALF, DIM)

    Ar = Gr @ E
    Ai = Gi @ E
    f8 = ml_dtypes.float8_e4m3
    trio = np.stack([Ar, Ar + Ai, Ai - Ar], axis=0) * W_SCALE   # [3, 2048, 4096]

    # wg[ktp, p, jt, kt2, fam, f] = trio[fam, ks*1024+(ktp*2+kt2)*128+f, jt*128+p]
    t7 = trio.reshape(3, NKS, KTP, 2, 128, NT, 128)  # [fam, ks, ktp, kt2, f, jt, p]
    wgs = []
    for ks in range(NKS):
        wg = np.ascontiguousarray(
            t7[:, ks].transpose(1, 5, 4, 2, 0, 3)).astype(f8)  # [ktp,p,jt,kt2,fam,f]
        wgs.append(wg.reshape(KTP, 128, NT * 2 * 3 * 128))

    # ut[p, jt, fam2(ur,ui), b] bf16 per batch shard
    bf = ml_dtypes.bfloat16
    fams = np.stack([ur, ui], axis=0)                # [2, B, 4096]
    famT = fams.transpose(0, 2, 1).reshape(2, NT, 128, B)
    utabs = []
    for bs in range(NBS):
        sl = slice(bs * BLOC, (bs + 1) * BLOC)
        t = famT[:, :, :, sl]                         # [2, jt, p, b]
        t = np.ascontiguousarray(t.transpose(2, 1, 0, 3)).astype(bf)
        utabs.append(t.reshape(128, NT * 2 * BLOC))
    return wgs, utabs


def _build_module():
    import concourse.tile as tile
    import concourse.mybir as mybir
    from concourse import bacc

    f32 = mybir.dt.float32
    bf16 = mybir.dt.bfloat16
    f8 = mybir.dt.float8e4

    nc = bacc.Bacc("TRN2", target_bir_lowering=False, debug=False)
    wg_ap = nc.dram_tensor("wg", [KTP, 128, NT * 2 * 3 * 128], f8,
                           kind="ExternalInput").ap()
    ut_ap = nc.dram_tensor("ut", [128, NT * 2 * BLOC], bf16,
                           kind="ExternalInput").ap()
    out_ap = nc.dram_tensor("out", [1, BLOC], f32, kind="ExternalOutput").ap()

    with tile.TileContext(nc) as tc:
        with ExitStack() as ctx:
            const = ctx.enter_context(tc.tile_pool(name="const", bufs=1))
            state = ctx.enter_context(tc.tile_pool(name="state", bufs=1))
            gpool = ctx.enter_context(tc.tile_pool(name="gpool", bufs=2))
            tmp = ctx.enter_context(tc.tile_pool(name="tmp", bufs=1))
            ps_mm = ctx.enter_context(tc.tile_pool(name="ps_mm", bufs=1, space="PSUM"))
            ps_out = ctx.enter_context(tc.tile_pool(name="ps_out", bufs=1, space="PSUM"))

            onesP = const.tile([128, 1], f32)
            nc.vector.memset(onesP[:], 1.0)

            ut = state.tile([128, NT, 2, BLOC], bf16)
            ust = state.tile([128, NT, BLOC], bf16)
            sqacc = state.tile([128, BLOC], f32)
            pso = ps_out.tile([1, BLOC], f32)

            # interleave u chunks with the first weight tile's chunks so the
            # PE can start as soon as chunk 0 of each lands; first chunks are
            # narrow (2 jt) to minimize time-to-first-matmul
            CH_SIZES = [2, 2, 4, 4, 4, 4, 4, 4, 4]
            CH_STARTS = [0, 2, 4, 8, 12, 16, 20, 24, 28]

            def issue_u_chunk(j0, jn):
                nc.sync.dma_start(
                    ut[:, j0:j0 + jn, :, :],
                    ut_ap[:, j0 * 2 * BLOC:(j0 + jn) * 2 * BLOC]
                    .rearrange("p (j f b) -> p j f b", j=jn, f=2))
                for t in range(j0, j0 + jn):
                    nc.vector.tensor_add(ust[:, t, :], ut[:, t, 0, :],
                                         ut[:, t, 1, :])

            def issue_w_chunk(gt, ktp, j0, jn):
                nc.sync.dma_start(
                    gt[:, j0:j0 + jn, :, :, :],
                    wg_ap[ktp, :, j0 * 2 * 3 * 128:(j0 + jn) * 2 * 3 * 128]
                    .rearrange("p (j k f w) -> p j k f w", j=jn, k=2, f=3))

            # startup: interleaved u / first-ktp weight chunks
            gt0 = gpool.tile([128, NT, 2, 3, 128], f8)
            for j0, jn in zip(CH_STARTS, CH_SIZES):
                issue_u_chunk(j0, jn)
                issue_w_chunk(gt0, 0, j0, jn)
            gts = [gt0]

            for ktp in range(KTP):
                if ktp == 0:
                    gt = gts[0]
                else:
                    gt = gpool.tile([128, NT, 2, 3, 128], f8)
                    for wc in range(8):
                        issue_w_chunk(gt, ktp, wc * 4, 4)
                pss = [[ps_mm.tile([128, BLOC], f32, name=f"ps_{k2}_{fm}",
                                   tag=f"ps_{k2}_{fm}")
                        for fm in range(3)] for k2 in range(2)]
                def mm3(jt, kt2):
                    st = (jt == 0)
                    sp = (jt == NT - 1)
                    ps1, ps2, ps3 = pss[kt2]
                    nc.tensor.matmul(ps1[:], gt[:, jt, kt2, 0, :],
                                     ust[:, jt, :], start=st, stop=sp)
                    nc.tensor.matmul(ps2[:], gt[:, jt, kt2, 1, :],
                                     ut[:, jt, 1, :], start=st, stop=sp)
                    nc.tensor.matmul(ps3[:], gt[:, jt, kt2, 2, :],
                                     ut[:, jt, 0, :], start=st, stop=sp)

                def drain(kt2, split):
                    ps1, ps2, ps3 = pss[kt2]
                    tk1 = tmp.tile([128, BLOC], f32, tag="k1", name="tk1")
                    twi = tmp.tile([128, BLOC], f32, tag="wi", name="twi")
                    tsq1 = tmp.tile([128, BLOC], f32, tag="sq1", name="tsq1")
                    tsq2 = tmp.tile([128, BLOC], f32, tag="sq2", name="tsq2")
                    if not split:
                        nc.vector.tensor_copy(tk1[:], ps1[:])
                        nc.vector.tensor_add(twi[:], tk1[:], ps3[:])
                        nc.vector.tensor_sub(tk1[:], tk1[:], ps2[:])
                        nc.scalar.activation(tsq1[:], tk1[:],
                                             mybir.ActivationFunctionType.Square)
                        nc.scalar.activation(tsq2[:], twi[:],
                                             mybir.ActivationFunctionType.Square)
                    else:
                        # engine-split critical path for the very last drain
                        nc.scalar.copy(tk1[:], ps1[:])
                        nc.vector.tensor_sub(tsq1[:], tk1[:], ps2[:])
                        nc.vector.tensor_add(twi[:], tk1[:], ps3[:])
                        nc.scalar.activation(tsq1[:], tsq1[:],
                                             mybir.ActivationFunctionType.Square)
                        nc.vector.tensor_mul(tsq2[:], twi[:], twi[:])
                    if ktp < KTP - 1:
                        if ktp == 0 and kt2 == 0:
                            nc.vector.tensor_add(sqacc[:], tsq1[:], tsq2[:])
                        else:
                            nc.vector.tensor_add(sqacc[:], sqacc[:], tsq1[:])
                            nc.vector.tensor_add(sqacc[:], sqacc[:], tsq2[:])
                    else:
                        nc.tensor.matmul(pso[:], onesP[:], tsq1[:],
                                         start=False, stop=False)
                        nc.tensor.matmul(pso[:], onesP[:], tsq2[:],
                                         start=False, stop=(kt2 == 1))

                last = (ktp == KTP - 1)
                jt_common = NT - 2 if last else NT
                for jt in range(jt_common):
                    for kt2 in range(2):
                        mm3(jt, kt2)
                if not last:
                    for kt2 in range(2):
                        drain(kt2, split=False)
                else:
                    # stagger: finish kt2=0's accumulation first so its drain
                    # overlaps kt2=1's final matmuls; flush sqacc first so the
                    # pso accumulation group starts before the tiny matmuls
                    for jt in range(NT - 2, NT):
                        mm3(jt, 0)
                    nc.tensor.matmul(pso[:], onesP[:], sqacc[:],
                                     start=True, stop=False)
                    drain(0, split=False)
                    for jt in range(NT - 2, NT):
                        mm3(jt, 1)
                    drain(1, split=True)

            osb = const.tile([1, BLOC], f32)
            nc.vector.tensor_copy(osb[:], pso[:])
            nc.sync.dma_start(out_ap[:], osb[:])

    nc.compile()
    return nc


def _get_module():
    global _BUILT
    if _BUILT is None:
        _BUILT = _build_module()
    return _BUILT


def kernel(inputs, weight, entangle_matrix, _trace=False, _tmpdir=None):
    from concourse.bass_utils import run_bass_kernel_spmd

    wgs, utabs = _host_prep(inputs, weight, entangle_matrix)
    nc = _get_module()

    if _trace:
        import jax
        jax.devices()

    in_maps = []
    for bs in range(NBS):
        for ks in range(NKS):
            in_maps.append({"wg": wgs[ks], "ut": utabs[bs]})
    res = run_bass_kernel_spmd(nc, in_maps, core_ids=list(range(NCORES)),
                               trace=_trace, tmpdir=_tmpdir)
    parts = [res.results[cix]["out"][0] for cix in range(NCORES)]
    out = np.concatenate([parts[bs * NKS + 0] + parts[bs * NKS + 1]
                          for bs in range(NBS)])
    out = out.astype(np.float32) * np.float32(1.0 / (W_SCALE * W_SCALE))
    if _trace:
        kernel.last_exec_time_ns = res.exec_time_ns
        kernel.last_profile = res
    return out



# revision 7
# speedup vs baseline: 1.5684x; 1.5684x over previous
"""Trainium2 Bass kernel v8: fp8 DoubleRow matmuls.

Gauss 3-mult complex matmul with BOTH operands fp8e4 and
perf_mode=DoubleRow (2 contraction slabs per matmul, ~1.5-1.8x the
bf16 MACs/cycle). Sharding 2 k-shards x 4 batch-shards; per core the
3x [1024, 4096] weight families and 3x [4096, 512] activation
families are fully SBUF-resident, PSUM accumulation groups are
double-buffered across the 8 output-row tiles, and squares are
reduced via DVE/ACT into a ones-matmul PSUM bank.
"""

import numpy as np
import ml_dtypes
from contextlib import ExitStack

N_QUBITS = 12
DIM = 4096
HALF = 2048
B = 2048
NCORES = 8
NKS = 2                     # k shards (A-row shards)
NBS = 4                     # batch shards
KLOC = HALF // NKS          # 1024 A rows per core
BLOC = B // NBS             # 512 batch cols per core
KT = KLOC // 128            # 8 output row tiles
KP = DIM // 256             # 16 contraction slab-pairs
FAM = 3
F8_MAX = 239.0              # ml_dtypes.float8_e4m3 max normal ~240

# fixed random signs for the H*D*H contraction-basis rotation
_DSIGN = (np.random.RandomState(12345).randint(0, 2, DIM) * 2 - 1)

_BUILT = None


def _butterfly(M):
    """Apply (2^-6)*(⊗12 [[1,-1],[1,1]]) along the last axis (4096)."""
    N = M.shape[0]
    T = M
    for q in range(N_QUBITS):
        T = T.reshape(N, 1 << q, 2, 1 << (N_QUBITS - 1 - q))
        a = T[:, :, 0, :]
        b = T[:, :, 1, :]
        T = np.stack([a - b, a + b], axis=2)
    return T.reshape(N, DIM) * np.asarray(2.0 ** -6, dtype=M.real.dtype)


def _mix(M):
    """Orthogonal flattening rotation O = H*D*H applied to rows of M.

    Applied identically to the A rows and the state vectors, it leaves
    A@u exact while making the u entries Gaussian-like, which is what
    fp8 quantization of the activations needs (raw u entries are
    Kronecker products with a few dominant entries that otherwise
    dominate the quadratic-form error)."""
    D = _DSIGN.astype(M.real.dtype)
    return _butterfly(_butterfly(M) * D)


def _host_prep(inputs, weight, entangle_matrix):
    x = np.asarray(inputs, dtype=np.float32)
    w = np.asarray(weight, dtype=np.float32)
    E = np.asarray(entangle_matrix, dtype=np.float32)

    ry = x / 2.0
    rz = (x * x) / 2.0
    a = np.cos(ry) * np.exp(-1j * rz)
    bq = np.sin(ry) * np.exp(1j * rz)
    col2 = np.stack([a, bq], axis=-1).astype(np.complex64)

    u = np.ones((B, 1), np.complex64)
    for q in range(N_QUBITS):
        u = (u[:, :, None] * col2[:, q][:, None, :]).reshape(B, -1)

    wr = w[3:]
    tx = wr[:N_QUBITS] / 2.0
    tz = wr[N_QUBITS:] / 2.0
    c, s = np.cos(tx), np.sin(tx)
    rx = np.stack([np.stack([c, -1j * s], -1), np.stack([-1j * s, c], -1)], -2)
    ez = np.exp(-1j * tz)
    zz = np.zeros_like(ez)
    rzm = np.stack([np.stack([ez, zz], -1), np.stack([zz, np.exp(1j * tz)], -1)], -2)
    mats = np.einsum('qij,qjk->qik', rx, rzm)

    def kron_list(ms):
        M = ms[0]
        for m_ in ms[1:]:
            M = np.kron(M, m_)
        return M

    RA = kron_list([mats[q] for q in range(0, 5)]).astype(np.complex64)
    RB = kron_list([mats[q] for q in range(5, 12)]).astype(np.complex64)

    def ry2(t):
        a_ = t / 2.0
        return np.array([[np.cos(a_), -np.sin(a_)], [np.sin(a_), np.cos(a_)]],
                        dtype=np.float32)

    rot = ry2(w[2]) @ ry2(w[1]) @ ry2(w[0])
    Etil = rot[0, 0] * E[:HALF, :] + rot[0, 1] * E[HALF:, :]

    E3 = Etil.reshape(HALF, 32, 128)
    Tr = (E3.reshape(-1, 128) @ RB.real).reshape(HALF, 32, 128)
    Ti = (E3.reshape(-1, 128) @ RB.imag).reshape(HALF, 32, 128)
    RAr, RAi = RA.real.astype(np.float32), RA.imag.astype(np.float32)
    Gr = (np.einsum('khL,hH->kHL', Tr, RAr)
          - np.einsum('khL,hH->kHL', Ti, RAi)).reshape(HALF, DIM)
    Gi = (np.einsum('khL,hH->kHL', Tr, RAi)
          + np.einsum('khL,hH->kHL', Ti, RAr)).reshape(HALF, DIM)

    Ar = _mix(Gr @ E)
    Ai = _mix(Gi @ E)
    um = _mix(u)
    ur = np.ascontiguousarray(um.real)
    ui = np.ascontiguousarray(um.imag)
    f8 = ml_dtypes.float8_e4m3

    trio = np.stack([Ar, Ar + Ai, Ai - Ar], axis=0)             # [3, 2048, 4096]
    af = np.stack([ur + ui, ui, ur], axis=0)                    # [3, 2048, 4096]
    w_scale = 2.0 ** np.floor(np.log2(F8_MAX / np.abs(trio).max()))
    u_scale = 2.0 ** np.floor(np.log2(F8_MAX / np.abs(af).max()))
    trio *= np.float32(w_scale)
    af *= np.float32(u_scale)

    # wgs[ks][p, kt, kp, fam, slab, m]
    wt7 = trio.reshape(FAM, NKS, KT, 128, KP, 2, 128)
    wgs = []
    for ks in range(NKS):
        wg = np.ascontiguousarray(
            wt7[:, ks].transpose(5, 1, 3, 0, 4, 2)).astype(f8)
        wgs.append(wg.reshape(128, KT * KP * FAM * 2 * 128))

    # uts[bs][p, kp, fam, slab, n]
    at6 = af.transpose(0, 2, 1).reshape(FAM, KP, 2, 128, NBS, BLOC)
    uts = []
    for bs in range(NBS):
        ut = np.ascontiguousarray(
            at6[:, :, :, :, bs, :].transpose(3, 1, 0, 2, 4)).astype(f8)
        uts.append(ut.reshape(128, KP * FAM * 2 * BLOC))
    return wgs, uts, 1.0 / (w_scale * w_scale * u_scale * u_scale)


def _build_module():
    import concourse.tile as tile
    import concourse.mybir as mybir
    from concourse import bacc

    f32 = mybir.dt.float32
    bf16 = mybir.dt.bfloat16
    f8 = mybir.dt.float8e4
    DR = mybir.MatmulPerfMode.DoubleRow
    SQ = mybir.ActivationFunctionType.Square

    nc = bacc.Bacc("TRN2", target_bir_lowering=False, debug=False)
    wg_ap = nc.dram_tensor("wg", [128, KT * KP * FAM * 2 * 128], f8,
                           kind="ExternalInput").ap()
    ut_ap = nc.dram_tensor("ut", [128, KP * FAM * 2 * BLOC], f8,
                           kind="ExternalInput").ap()
    out_ap = nc.dram_tensor("out", [1, BLOC], f32, kind="ExternalOutput").ap()

    WKP = FAM * 2 * 128            # weight bytes per (kt, kp) per partition
    UKP = FAM * 2 * BLOC           # act bytes per kp per partition

    with tile.TileContext(nc) as tc:
        with ExitStack() as ctx:
            const = ctx.enter_context(tc.tile_pool(name="const", bufs=1))
            state = ctx.enter_context(tc.tile_pool(name="state", bufs=1))
            tmp = ctx.enter_context(tc.tile_pool(name="tmp", bufs=1))
            ps_mm = ctx.enter_context(tc.tile_pool(name="ps_mm", bufs=2,
                                                   space="PSUM"))
            ps_out = ctx.enter_context(tc.tile_pool(name="ps_out", bufs=1,
                                                    space="PSUM"))

            onesF = const.tile([128, 1], f32)
            nc.vector.memset(onesF[:], 1.0)
            onesB = const.tile([128, 1], bf16)
            nc.vector.memset(onesB[:], 1.0)

            wsb = state.tile([128, KT, KP, FAM, 2, 128], f8)
            usb = state.tile([128, KP, FAM, 2, BLOC], f8)
            sqacc = state.tile([128, BLOC], f32)
            pso = ps_out.tile([1, BLOC], f32)

            def u_chunk(kp0, nkp):
                nc.sync.dma_start(
                    usb[:, kp0:kp0 + nkp, :, :, :],
                    ut_ap[:, kp0 * UKP:(kp0 + nkp) * UKP]
                    .rearrange("p (a f s n) -> p a f s n", a=nkp, f=FAM, s=2))

            def w_chunk(kt, kp0, nkp):
                off = (kt * KP + kp0) * WKP
                nc.sync.dma_start(
                    wsb[:, kt, kp0:kp0 + nkp, :, :, :],
                    wg_ap[:, off:off + nkp * WKP]
                    .rearrange("p (a f s m) -> p a f s m", a=nkp, f=FAM, s=2))

            # startup: fine-grained interleave, then bulk
            nc.sync.dma_start(
                usb[:, 0, 0, :, :],
                ut_ap[:, 0:2 * BLOC].rearrange("p (s n) -> p s n", s=2))
            w_chunk(0, 0, 1)
            nc.sync.dma_start(
                usb[:, 0, 1:3, :, :],
                ut_ap[:, 2 * BLOC:UKP]
                .rearrange("p (f s n) -> p f s n", f=2, s=2))
            w_chunk(0, 1, 1)
            u_chunk(1, 1)
            w_chunk(0, 2, 2)
            u_chunk(2, 2)
            w_chunk(0, 4, 4)
            u_chunk(4, 4)
            w_chunk(0, 8, 8)
            u_chunk(8, 8)
            w_chunk(1, 0, 8)
            w_chunk(1, 8, 8)
            for kt in range(2, KT):
                w_chunk(kt, 0, 16)

            for kt in range(KT):
                last = (kt == KT - 1)
                pss = [ps_mm.tile([128, BLOC], f32, name=f"ps_{fm}",
                                  tag=f"ps_{fm}") for fm in range(FAM)]
                if last:
                    # open the output accumulation group early so the
                    # (slow) f32 sqacc flush hides under this kt's MMs
                    nc.tensor.matmul(pso[:], onesF[:], sqacc[:],
                                     start=True, stop=False)
                for kp in range(KP):
                    for fm in range(FAM):
                        nc.tensor.matmul(pss[fm][:],
                                         wsb[:, kt, kp, fm, :, :],
                                         usb[:, kp, fm, :, :],
                                         start=(kp == 0), stop=(kp == KP - 1),
                                         perf_mode=DR)
                ps1, ps2, ps3 = pss
                if not last:
                    tk1 = tmp.tile([128, BLOC], f32, tag="tk1", name="tk1")
                    twi = tmp.tile([128, BLOC], f32, tag="twi", name="twi")
                    tsq1 = tmp.tile([128, BLOC], f32, tag="tsq1", name="tsq1")
                    tsq2 = tmp.tile([128, BLOC], f32, tag="tsq2", name="tsq2")
                    nc.vector.tensor_copy(tk1[:], ps1[:])
                    nc.vector.tensor_add(twi[:], tk1[:], ps3[:])
                    nc.vector.tensor_sub(tk1[:], tk1[:], ps2[:])
                    nc.scalar.activation(tsq1[:], tk1[:], SQ)
                    nc.scalar.activation(tsq2[:], twi[:], SQ)
                    if kt == 0:
                        nc.vector.tensor_add(sqacc[:], tsq1[:], tsq2[:])
                    else:
                        nc.vector.tensor_add(sqacc[:], sqacc[:], tsq1[:])
                        nc.vector.tensor_add(sqacc[:], sqacc[:], tsq2[:])
                else:
                    # engine-split tail: ACT copies, DVE sub/add, bf16
                    # squares feed two short bf16 ones-matmuls
                    tk1b = tmp.tile([128, BLOC], bf16, tag="tk1b", name="tk1b")
                    twib = tmp.tile([128, BLOC], bf16, tag="twib", name="twib")
                    sq1b = tmp.tile([128, BLOC], bf16, tag="sq1b", name="sq1b")
                    sq2b = tmp.tile([128, BLOC], bf16, tag="sq2b", name="sq2b")
                    nc.scalar.copy(tk1b[:], ps1[:])
                    nc.vector.tensor_sub(sq1b[:], tk1b[:], ps2[:])
                    nc.vector.tensor_add(twib[:], tk1b[:], ps3[:])
                    nc.scalar.activation(sq1b[:], sq1b[:], SQ)
                    nc.vector.tensor_mul(sq2b[:], twib[:], twib[:])
                    nc.tensor.matmul(pso[:], onesB[:], sq1b[:],
                                     start=False, stop=False)
                    nc.tensor.matmul(pso[:], onesB[:], sq2b[:],
                                     start=False, stop=True)

            osb = const.tile([1, BLOC], f32)
            nc.vector.tensor_copy(osb[:], pso[:])
            nc.sync.dma_start(out_ap[:], osb[:])

    nc.compile()
    return nc


def _get_module():
    global _BUILT
    if _BUILT is None:
        _BUILT = _build_module()
    return _BUILT


def kernel(inputs, weight, entangle_matrix, _trace=False, _tmpdir=None):
    from concourse.bass_utils import run_bass_kernel_spmd

    wgs, uts, out_scale = _host_prep(inputs, weight, entangle_matrix)
    nc = _get_module()

    if _trace:
        import jax
        jax.devices()

    in_maps = []
    for bs in range(NBS):
        for ks in range(NKS):
            in_maps.append({"wg": wgs[ks], "ut": uts[bs]})
    res = run_bass_kernel_spmd(nc, in_maps, core_ids=list(range(NCORES)),
                               trace=_trace, tmpdir=_tmpdir)
    parts = [res.results[cix]["out"][0] for cix in range(NCORES)]
    out = np.concatenate([parts[bs * NKS + 0] + parts[bs * NKS + 1]
                          for bs in range(NBS)])
    out = out.astype(np.float32) * np.float32(out_scale)
    if _trace:
        kernel.last_exec_time_ns = res.exec_time_ns
        kernel.last_profile = res
    return out


# revision 10
# speedup vs baseline: 1.7115x; 1.0912x over previous
"""Trainium2 Bass kernel v9: fp8 DoubleRow matmuls, DMA-paced startup fix.

Gauss 3-mult complex matmul with BOTH operands fp8e4 and
perf_mode=DoubleRow. Host applies an orthogonal H*D*H rotation to the
contraction basis (exact for A@u) so fp8 activation quantization sees
Gaussian-like entries instead of spiky Kronecker products.

Device: 2 k-shards x 4 batch-shards. The first two output row-tiles
(kt0, kt1) are swept kp-major interleaved so early compute is paced by
half the activation DMA bandwidth demand; kt2..kt7 run serially with
PSUM bank rotation. Squares reduce via DVE/ACT + ones-matmul.
"""

import numpy as np
import ml_dtypes
from contextlib import ExitStack

N_QUBITS = 12
DIM = 4096
HALF = 2048
B = 2048
NCORES = 8
NKS = 2                     # k shards (A-row shards)
NBS = 4                     # batch shards
KLOC = HALF // NKS          # 1024 A rows per core
BLOC = B // NBS             # 512 batch cols per core
KT = KLOC // 128            # 8 output row tiles
PR = KT // 2                # 4 row-tile pairs
KP = DIM // 256             # 16 contraction slab-pairs
FAM = 3
F8_MAX = 239.0              # ml_dtypes.float8_e4m3 max normal ~240

# fixed random signs for the H*D*H contraction-basis rotation
_DSIGN = (np.random.RandomState(12345).randint(0, 2, DIM) * 2 - 1)

_BUILT = None


def _butterfly(M):
    """Apply (2^-6)*(⊗12 [[1,-1],[1,1]]) along the last axis (4096)."""
    N = M.shape[0]
    T = M
    for q in range(N_QUBITS):
        T = T.reshape(N, 1 << q, 2, 1 << (N_QUBITS - 1 - q))
        a = T[:, :, 0, :]
        b = T[:, :, 1, :]
        T = np.stack([a - b, a + b], axis=2)
    return T.reshape(N, DIM) * np.asarray(2.0 ** -6, dtype=M.real.dtype)


def _mix(M):
    """Orthogonal flattening rotation O = H*D*H applied to rows of M.

    Applied identically to A's rows and the state vectors it leaves
    A@u exact while making the state entries Gaussian-like, which fp8
    activation quantization needs (raw Kronecker-product states have a
    few dominant entries that dominate the quadratic-form error)."""
    D = _DSIGN.astype(M.real.dtype)
    return _butterfly(_butterfly(M) * D)


def _host_prep(inputs, weight, entangle_matrix):
    x = np.asarray(inputs, dtype=np.float32)
    w = np.asarray(weight, dtype=np.float32)
    E = np.asarray(entangle_matrix, dtype=np.float32)

    ry = x / 2.0
    rz = (x * x) / 2.0
    a = np.cos(ry) * np.exp(-1j * rz)
    bq = np.sin(ry) * np.exp(1j * rz)
    col2 = np.stack([a, bq], axis=-1).astype(np.complex64)

    u = np.ones((B, 1), np.complex64)
    for q in range(N_QUBITS):
        u = (u[:, :, None] * col2[:, q][:, None, :]).reshape(B, -1)

    wr = w[3:]
    tx = wr[:N_QUBITS] / 2.0
    tz = wr[N_QUBITS:] / 2.0
    c, s = np.cos(tx), np.sin(tx)
    rx = np.stack([np.stack([c, -1j * s], -1), np.stack([-1j * s, c], -1)], -2)
    ez = np.exp(-1j * tz)
    zz = np.zeros_like(ez)
    rzm = np.stack([np.stack([ez, zz], -1), np.stack([zz, np.exp(1j * tz)], -1)], -2)
    mats = np.einsum('qij,qjk->qik', rx, rzm)

    def kron_list(ms):
        M = ms[0]
        for m_ in ms[1:]:
            M = np.kron(M, m_)
        return M

    RA = kron_list([mats[q] for q in range(0, 5)]).astype(np.complex64)
    RB = kron_list([mats[q] for q in range(5, 12)]).astype(np.complex64)

    def ry2(t):
        a_ = t / 2.0
        return np.array([[np.cos(a_), -np.sin(a_)], [np.sin(a_), np.cos(a_)]],
                        dtype=np.float32)

    rot = ry2(w[2]) @ ry2(w[1]) @ ry2(w[0])
    Etil = rot[0, 0] * E[:HALF, :] + rot[0, 1] * E[HALF:, :]

    E3 = Etil.reshape(HALF, 32, 128)
    Tr = (E3.reshape(-1, 128) @ RB.real).reshape(HALF, 32, 128)
    Ti = (E3.reshape(-1, 128) @ RB.imag).reshape(HALF, 32, 128)
    RAr, RAi = RA.real.astype(np.float32), RA.imag.astype(np.float32)
    Gr = (np.einsum('khL,hH->kHL', Tr, RAr)
          - np.einsum('khL,hH->kHL', Ti, RAi)).reshape(HALF, DIM)
    Gi = (np.einsum('khL,hH->kHL', Tr, RAi)
          + np.einsum('khL,hH->kHL', Ti, RAr)).reshape(HALF, DIM)

    Ar = _mix(Gr @ E)
    Ai = _mix(Gi @ E)
    um = _mix(u)
    ur = np.ascontiguousarray(um.real)
    ui = np.ascontiguousarray(um.imag)
    f8 = ml_dtypes.float8_e4m3

    trio = np.stack([Ar, Ar + Ai, Ai - Ar], axis=0)             # [3, 2048, 4096]
    af = np.stack([ur + ui, ui, ur], axis=0)                    # [3, 2048, 4096]
    w_scale = 2.0 ** np.floor(np.log2(F8_MAX / np.abs(trio).max()))
    u_scale = 2.0 ** np.floor(np.log2(F8_MAX / np.abs(af).max()))
    trio *= np.float32(w_scale)
    af *= np.float32(u_scale)

    # wgs[ks][p, pr, kp, kt2, fam, slab, m]  (A row = ks*1024 + (pr*2+kt2)*128 + m,
    #                                         contraction j = kp*256 + slab*128 + p)
    wt8 = trio.reshape(FAM, NKS, PR, 2, 128, KP, 2, 128)
    wgs = []
    for ks in range(NKS):
        wg = np.ascontiguousarray(
            wt8[:, ks].transpose(6, 1, 4, 2, 0, 5, 3)).astype(f8)
        wgs.append(wg.reshape(128, KT * KP * FAM * 2 * 128))

    # uts[bs][p, kp, fam, slab, n]
    at6 = af.transpose(0, 2, 1).reshape(FAM, KP, 2, 128, NBS, BLOC)
    uts = []
    for bs in range(NBS):
        ut = np.ascontiguousarray(
            at6[:, :, :, :, bs, :].transpose(3, 1, 0, 2, 4)).astype(f8)
        uts.append(ut.reshape(128, KP * FAM * 2 * BLOC))
    return wgs, uts, 1.0 / (w_scale * w_scale * u_scale * u_scale)


def _build_module():
    import concourse.tile as tile
    import concourse.mybir as mybir
    from concourse import bacc

    f32 = mybir.dt.float32
    bf16 = mybir.dt.bfloat16
    f8 = mybir.dt.float8e4
    DR = mybir.MatmulPerfMode.DoubleRow
    SQ = mybir.ActivationFunctionType.Square

    nc = bacc.Bacc("TRN2", target_bir_lowering=False, debug=False)
    wg_ap = nc.dram_tensor("wg", [128, KT * KP * FAM * 2 * 128], f8,
                           kind="ExternalInput").ap()
    ut_ap = nc.dram_tensor("ut", [128, KP * FAM * 2 * BLOC], f8,
                           kind="ExternalInput").ap()
    out_ap = nc.dram_tensor("out", [1, BLOC], f32, kind="ExternalOutput").ap()

    WKP = 2 * FAM * 2 * 128        # weight bytes per (pr, kp) per partition
    UKP = FAM * 2 * BLOC           # act bytes per kp per partition

    with tile.TileContext(nc) as tc:
        with ExitStack() as ctx:
            const = ctx.enter_context(tc.tile_pool(name="const", bufs=1))
            state = ctx.enter_context(tc.tile_pool(name="state", bufs=1))
            tmp = ctx.enter_context(tc.tile_pool(name="tmp", bufs=1))
            ps_mm = ctx.enter_context(tc.tile_pool(name="ps_mm", bufs=1,
                                                   space="PSUM"))
            ps_out = ctx.enter_context(tc.tile_pool(name="ps_out", bufs=1,
                                                    space="PSUM"))

            onesF = const.tile([128, 1], f32)
            nc.vector.memset(onesF[:], 1.0)
            onesB = const.tile([128, 1], bf16)
            nc.vector.memset(onesB[:], 1.0)

            wsb = state.tile([128, PR, KP, 2, FAM, 2, 128], f8)
            usb = state.tile([128, KP, FAM, 2, BLOC], f8)
            sqacc = state.tile([128, BLOC], f32)
            pso = ps_out.tile([1, BLOC], f32)

            def u_chunk(kp0, nkp):
                nc.sync.dma_start(
                    usb[:, kp0:kp0 + nkp, :, :, :],
                    ut_ap[:, kp0 * UKP:(kp0 + nkp) * UKP]
                    .rearrange("p (a f s n) -> p a f s n", a=nkp, f=FAM, s=2))

            def w_chunk(pr, kp0, nkp):
                off = (pr * KP + kp0) * WKP
                nc.sync.dma_start(
                    wsb[:, pr, kp0:kp0 + nkp, :, :, :, :],
                    wg_ap[:, off:off + nkp * WKP]
                    .rearrange("p (a k f s m) -> p a k f s m",
                               a=nkp, k=2, f=FAM, s=2))

            # DMA issue in consumption order; first chunks small
            u_chunk(0, 1)
            w_chunk(0, 0, 1)
            u_chunk(1, 1)
            w_chunk(0, 1, 1)
            u_chunk(2, 2)
            w_chunk(0, 2, 2)
            u_chunk(4, 2)
            w_chunk(0, 4, 2)
            u_chunk(6, 2)
            w_chunk(0, 6, 2)
            u_chunk(8, 4)
            w_chunk(0, 8, 4)
            u_chunk(12, 4)
            w_chunk(0, 12, 4)
            for pr in range(1, PR):
                w_chunk(pr, 0, 8)
                w_chunk(pr, 8, 8)

            def mm3(ps, pr, kp, kt2, start, stop):
                for fm in range(FAM):
                    nc.tensor.matmul(ps[fm][:],
                                     wsb[:, pr, kp, kt2, fm, :, :],
                                     usb[:, kp, fm, :, :],
                                     start=start, stop=stop,
                                     perf_mode=DR)

            def drain(ps, first):
                ps1, ps2, ps3 = ps
                tk1 = tmp.tile([128, BLOC], f32, tag="tk1", name="tk1")
                twi = tmp.tile([128, BLOC], f32, tag="twi", name="twi")
                tsq1 = tmp.tile([128, BLOC], f32, tag="tsq1", name="tsq1")
                tsq2 = tmp.tile([128, BLOC], f32, tag="tsq2", name="tsq2")
                nc.vector.tensor_copy(tk1[:], ps1[:])
                nc.vector.tensor_add(twi[:], tk1[:], ps3[:])
                nc.vector.tensor_sub(tk1[:], tk1[:], ps2[:])
                nc.scalar.activation(tsq1[:], tk1[:], SQ)
                nc.scalar.activation(tsq2[:], twi[:], SQ)
                if first:
                    nc.vector.tensor_add(sqacc[:], tsq1[:], tsq2[:])
                else:
                    nc.vector.tensor_add(sqacc[:], sqacc[:], tsq1[:])
                    nc.vector.tensor_add(sqacc[:], sqacc[:], tsq2[:])

            def drain_tail(ps):
                # half-tile pipelined across ACT (PSUM copy, square) and
                # DVE (sub/add/mul) to shorten the post-last-matmul chain
                ps1, ps2, ps3 = ps
                tk1b = tmp.tile([128, BLOC], bf16, tag="tk1b", name="tk1b")
                twib = tmp.tile([128, BLOC], bf16, tag="twib", name="twib")
                sq1b = tmp.tile([128, BLOC], bf16, tag="sq1b", name="sq1b")
                sq2b = tmp.tile([128, BLOC], bf16, tag="sq2b", name="sq2b")
                nh = 2
                hw_ = BLOC // nh
                for h in range(nh):
                    sl = slice(h * hw_, (h + 1) * hw_)
                    nc.scalar.copy(tk1b[:, sl], ps1[:, sl])
                    nc.vector.tensor_sub(sq1b[:, sl], tk1b[:, sl], ps2[:, sl])
                    nc.vector.tensor_add(twib[:, sl], tk1b[:, sl], ps3[:, sl])
                    nc.vector.tensor_mul(sq1b[:, sl], sq1b[:, sl], sq1b[:, sl])
                    nc.scalar.activation(sq2b[:, sl], twib[:, sl], SQ)
                    nc.tensor.matmul(pso[:, sl], onesB[:], sq1b[:, sl],
                                     start=False, stop=False)
                    nc.tensor.matmul(pso[:, sl], onesB[:], sq2b[:, sl],
                                     start=False, stop=True)

            psA = [ps_mm.tile([128, BLOC], f32, name=f"psA_{fm}",
                              tag=f"psA_{fm}") for fm in range(FAM)]
            psB = [ps_mm.tile([128, BLOC], f32, name=f"psB_{fm}",
                              tag=f"psB_{fm}") for fm in range(FAM)]
            # pair 0: kt0/kt1 interleaved kp-major (halves early act demand)
            for kp in range(KP):
                mm3(psA, 0, kp, 0, kp == 0, kp == KP - 1)
                mm3(psB, 0, kp, 1, kp == 0, kp == KP - 1)
            drain(psA, first=True)
            drain(psB, first=False)

            for kt in range(2, KT):
                pr, kt2 = divmod(kt, 2)
                tagset = "AB"[kt & 1]
                ps = [ps_mm.tile([128, BLOC], f32, name=f"ps{tagset}_{fm}",
                                 tag=f"ps{tagset}_{fm}") for fm in range(FAM)]
                last = (kt == KT - 1)
                if last:
                    # open the output group early; slow f32 flush hides
                    # under this kt's matmuls
                    nc.tensor.matmul(pso[:], onesF[:], sqacc[:],
                                     start=True, stop=False)
                for kp in range(KP):
                    mm3(ps, pr, kp, kt2, kp == 0, kp == KP - 1)
                if not last:
                    drain(ps, first=False)
                else:
                    drain_tail(ps)

            osb = const.tile([1, BLOC], f32)
            nc.vector.tensor_copy(osb[:], pso[:])
            nc.sync.dma_start(out_ap[:], osb[:])

    nc.compile()
    return nc


def _get_module():
    global _BUILT
    if _BUILT is None:
        _BUILT = _build_module()
    return _BUILT


def kernel(inputs, weight, entangle_matrix, _trace=False, _tmpdir=None):
    from concourse.bass_utils import run_bass_kernel_spmd

    wgs, uts, out_scale = _host_prep(inputs, weight, entangle_matrix)
    nc = _get_module()

    if _trace:
        import jax
        jax.devices()

    in_maps = []
    for bs in range(NBS):
        for ks in range(NKS):
            in_maps.append({"wg": wgs[ks], "ut": uts[bs]})
    res = run_bass_kernel_spmd(nc, in_maps, core_ids=list(range(NCORES)),
                               trace=_trace, tmpdir=_tmpdir)
    parts = [res.results[cix]["out"][0] for cix in range(NCORES)]
    out = np.concatenate([parts[bs * NKS + 0] + parts[bs * NKS + 1]
                          for bs in range(NBS)])
    out = out.astype(np.float32) * np.float32(out_scale)
    if _trace:
        kernel.last_exec_time_ns = res.exec_time_ns
        kernel.last_profile = res
    return out
